# revision 1
# baseline (speedup 1.0000x reference)
"""Trainium2 Bass kernel for DiffusionLoss (L1 noise loss + chamfer distance).

Contract: kernel(**inputs) takes the FULL [8, 16384, 3] f32 inputs, shards the
batch across 8 NeuronCores (1 batch element per core), and returns the full
scalar loss (shape () float32).

Per-core computation (batch element b):
  noise_part = sum |pn - an|
  d_pt[i]    = min_j ||pred_i - targ_j||^2   (row mins)
  d_tp[j]    = min_i ||pred_i - targ_j||^2   (col mins)
  out[1,1]   = noise_part/(8*N*3) + 0.1/(8*N) * (sum relu(d_pt)+sum relu(d_tp))
Host sums the 8 partial scalars.

Execution on this target is dominated by per-instruction dispatch (engine
timelines are effectively serial), so the kernel minimizes INSTRUCTION COUNT:
the distance matrix is produced in [128, JW]-wide strips with fused
scalar_tensor_tensor ops instead of PE matmul tiles.

Layout: partition p of an i-band t holds pred point i = 128*t + p; the free
axis spans JW target points j. Broadcast rows B_d[128, JW] hold targ coords
replicated across partitions, bsq_b holds |targ_j|^2. The y/z rows are bf16:
since bsq_b is computed FROM the quantized rows, D stays the exact geometric
distance to slightly-perturbed target points (no catastrophic cancellation;
the perturbation averages to ~1e-5 on the chamfer mean). bx, bsq_b and the
D accumulation remain f32.

Bands are processed in PAIRS sharing one [P, 2, JW] D tile (9 DVE ops/pair):
  for u in (0, 1):  # band t = 2q+u
    D_u = (B_x * -2a_x[i]) + bsq_b       # scalar_tensor_tensor, per-part scalar
    D_u = (B_y * -2a_y[i]) + D_u
    D_u = (B_z * -2a_z[i]) + D_u         # D[p,j] = |b_j|^2 - 2 a_i . b_j
  rowm[:, 2q:2q+2] = reduce_min_j(D)     # ONE 3D-AP reduce covers both bands;
                                         # |a_i|^2 added in batched epilogue
  colacc = min(colacc, D_u + |a_i|^2)    # fused add+min accumulate, per band

Col mins finish with one gpsimd partition_all_reduce(max) over the negated
accumulator. N=16384 needs two JW=8192 halves to fit the rows in SBUF.
Total: ~1.29k instructions per core (vs ~9.5k for a PE-matmul formulation).
"""

import numpy as np
from contextlib import ExitStack

import concourse.bacc as bacc
import concourse.bass as bass
import concourse.bass_isa as bass_isa
import concourse.mybir as mybir
import concourse.tile as tile
from concourse.bass_utils import run_bass_kernel_spmd

F32 = mybir.dt.float32
BF16 = mybir.dt.bfloat16
A = mybir.AluOpType
AX = mybir.AxisListType

B = 8
N = 16384
NCORES = 8
P = 128
BIG = 3.0e38
JW_MAX = 8192

NOISE_WEIGHT = 1.0
CHAMFER_WEIGHT = 0.1


def diffusion_loss_kernel(ctx, tc, out_ap, ins, n=N):
    nc = tc.nc
    assert n % P == 0
    nt = n // P
    npp = n // P
    jw = min(JW_MAX, n)
    nh = n // jw
    wn = float(NOISE_WEIGHT / (B * n * 3))
    wc = float(CHAMFER_WEIGHT / (B * n))

    consts = ctx.enter_context(tc.tile_pool(name="consts", bufs=1))

    # ---------------- noise L1 loss ----------------
    noiseacc = consts.tile([P, 1], F32)
    with tc.tile_pool(name="noise", bufs=1) as nprep:
        pn_nat = nprep.tile([P, 3 * npp], F32)
        an_nat = nprep.tile([P, 3 * npp], F32)
        nc.sync.dma_start(pn_nat[:], ins["pn"].rearrange("(p f) d -> p (f d)", p=P))
        nc.sync.dma_start(an_nat[:], ins["an"].rearrange("(p f) d -> p (f d)", p=P))
        nc.vector.tensor_sub(pn_nat[:], pn_nat[:], an_nat[:])
        nc.vector.tensor_reduce(
            noiseacc[:], pn_nat[:], axis=AX.X, op=A.add, apply_absolute_value=True
        )

    # ---------------- pred-side per-partition scalars ----------------
    # acols[p, d, t] = pred coord d of point i = 128*t + p  (then scaled by -2)
    acols = consts.tile([P, 3, nt], F32)
    pred_t = ins["pred"].rearrange("(t p) d -> p t d", p=P)
    for d in range(3):
        nc.sync.dma_start(acols[:, d, :], pred_t[:, :, d])
    asq = consts.tile([P, nt], F32)
    tmp = consts.tile([P, nt], F32)
    nc.vector.tensor_mul(asq[:], acols[:, 0, :], acols[:, 0, :])
    nc.vector.tensor_mul(tmp[:], acols[:, 1, :], acols[:, 1, :])
    nc.vector.tensor_add(asq[:], asq[:], tmp[:])
    nc.vector.tensor_mul(tmp[:], acols[:, 2, :], acols[:, 2, :])
    nc.vector.tensor_add(asq[:], asq[:], tmp[:])
    nasq = consts.tile([P, nt], F32)
    nc.vector.tensor_scalar_mul(nasq[:], asq[:], -1.0)
    nc.vector.tensor_scalar_mul(
        acols.rearrange("p a b -> p (a b)"), acols.rearrange("p a b -> p (a b)"),
        -2.0,
    )

    # ---------------- main strips ----------------
    bx = consts.tile([P, jw], F32)
    by = consts.tile([P, jw], BF16)
    bz = consts.tile([P, jw], BF16)
    bsq_b = consts.tile([P, jw], F32)
    dmat = consts.tile([P, 2, jw], F32)
    colacc = consts.tile([P, jw], BF16)
    rowm = consts.tile([P, nh, nt], F32)
    colsum = consts.tile([1, 1], F32)
    nc.vector.memzero(colsum[:])
    csum_h = consts.tile([1, 1], F32)

    for h in range(nh):
        jb = h * jw
        for d, bt in ((0, bx), (1, by), (2, bz)):
            if bt is bx:
                nc.sync.dma_start(
                    bt[0:1, :],
                    ins["targ"][jb : jb + jw, d : d + 1].rearrange("j o -> o j"),
                )
            else:
                nc.sync.dma_start(
                    dmat[0:1, 0, :],
                    ins["targ"][jb : jb + jw, d : d + 1].rearrange("j o -> o j"),
                )
                nc.vector.tensor_copy(bt[0:1, :], dmat[0:1, 0, :])
            nc.gpsimd.partition_broadcast(bt[:], bt[0:1, :], channels=P)
        # |b_j|^2 on row 0 (dmat row 0 as scratch), then broadcast
        nc.vector.tensor_mul(bsq_b[0:1, :], bx[0:1, :], bx[0:1, :])
        nc.vector.tensor_mul(dmat[0:1, 0, :], by[0:1, :], by[0:1, :])
        nc.vector.tensor_add(bsq_b[0:1, :], bsq_b[0:1, :], dmat[0:1, 0, :])
        nc.vector.tensor_mul(dmat[0:1, 0, :], bz[0:1, :], bz[0:1, :])
        nc.vector.tensor_add(bsq_b[0:1, :], bsq_b[0:1, :], dmat[0:1, 0, :])
        nc.gpsimd.partition_broadcast(bsq_b[:], bsq_b[0:1, :], channels=P)
        nc.vector.memset(colacc[:], BIG)

        for q in range(nt // 2):
            for u in range(2):
                t = 2 * q + u
                d_u = dmat[:, u, :]
                nc.vector.scalar_tensor_tensor(
                    out=d_u, in0=bx[:], scalar=acols[:, 0, t : t + 1],
                    in1=bsq_b[:], op0=A.mult, op1=A.add,
                )
                nc.vector.scalar_tensor_tensor(
                    out=d_u, in0=by[:], scalar=acols[:, 1, t : t + 1],
                    in1=d_u, op0=A.mult, op1=A.add,
                )
                nc.vector.scalar_tensor_tensor(
                    out=d_u, in0=bz[:], scalar=acols[:, 2, t : t + 1],
                    in1=d_u, op0=A.mult, op1=A.add,
                )
            # one reduce covers both bands (innermost-axis min on [P, 2, jw])
            nc.vector.tensor_reduce(
                rowm[:, h, 2 * q : 2 * q + 2], dmat[:], axis=AX.X, op=A.min
            )
            for u in range(2):
                t = 2 * q + u
                # colacc = min(colacc, D + |a_i|^2)  (subtract negated asq)
                nc.vector.scalar_tensor_tensor(
                    out=colacc[:], in0=dmat[:, u, :], scalar=nasq[:, t : t + 1],
                    in1=colacc[:], op0=A.subtract, op1=A.min,
                )

        # ---- col mins for this half ----
        nc.vector.tensor_scalar_mul(dmat[:, 0, :], colacc[:], -1.0)
        nc.gpsimd.partition_all_reduce(
            bsq_b[:], dmat[:, 0, :], channels=P, reduce_op=bass_isa.ReduceOp.max
        )
        # sum_j relu(colmin_j) = -sum_j min(-colmin_j, 0)
        nc.vector.tensor_scalar_min(dmat[0:1, 0, :], bsq_b[0:1, :], 0.0)
        nc.vector.tensor_reduce(csum_h[:], dmat[0:1, 0, :], axis=AX.X, op=A.add)
        nc.vector.tensor_sub(colsum[:], colsum[:], csum_h[:])

    # ---------------- row mins epilogue ----------------
    rfin = rowm[:, 0, :]
    for h in range(1, nh):
        nc.vector.tensor_tensor(out=rfin, in0=rfin, in1=rowm[:, h, :], op=A.min)
    nc.vector.tensor_add(rfin, rfin, asq[:])
    nc.vector.tensor_scalar_max(rfin, rfin, 0.0)
    rvec = consts.tile([P, 1], F32)
    nc.vector.tensor_reduce(rvec[:], rfin, axis=AX.X, op=A.add)

    # ---------------- combine ----------------
    nc.vector.tensor_scalar_mul(rvec[:], rvec[:], wc)
    v = consts.tile([P, 1], F32)
    nc.vector.scalar_tensor_tensor(
        out=v[:], in0=noiseacc[:], scalar=wn, in1=rvec[:], op0=A.mult, op1=A.add
    )
    ones_col = consts.tile([P, 1], F32)
    nc.vector.memset(ones_col[:], 1.0)
    with tc.tile_pool(name="eppsum", bufs=1, space="PSUM") as ep_psum:
        fin = ep_psum.tile([1, 1], F32)
        nc.tensor.matmul(fin[:], v[:], ones_col[:], start=True, stop=True)
        fs = consts.tile([1, 1], F32)
        nc.vector.tensor_copy(fs[:], fin[:])
    nc.vector.scalar_tensor_tensor(
        out=fs[:], in0=colsum[:], scalar=wc, in1=fs[:], op0=A.mult, op1=A.add
    )
    nc.sync.dma_start(out_ap, fs[:])


_CACHE = {}


def build_program(n=N):
    if n not in _CACHE:
        nc = bacc.Bacc(
            "TRN2", target_bir_lowering=False, debug=False, enable_asserts=False
        )
        ins = {
            name: nc.dram_tensor(name, [n, 3], F32, kind="ExternalInput").ap()
            for name in ("pn", "an", "pred", "targ")
        }
        out_ap = nc.dram_tensor("out", [1, 1], F32, kind="ExternalOutput").ap()
        with tile.TileContext(nc) as tc:
            with ExitStack() as ctx:
                diffusion_loss_kernel(ctx, tc, out_ap, ins, n=n)
        nc.compile()
        _CACHE[n] = nc
    return _CACHE[n]


def run_cores(inputs, n=N, trace=False):
    """Run the SPMD program over the batch; returns (partials, results)."""
    nc = build_program(n=n)
    pn = np.ascontiguousarray(np.asarray(inputs["predicted_noise"], np.float32))
    an = np.ascontiguousarray(np.asarray(inputs["actual_noise"], np.float32))
    pred = np.ascontiguousarray(
        np.asarray(inputs["predicted_points_coarse"], np.float32)
    )
    targ = np.ascontiguousarray(
        np.asarray(inputs["target_points_coarse"], np.float32)
    )
    in_maps = [
        {"pn": pn[b], "an": an[b], "pred": pred[b], "targ": targ[b]}
        for b in range(pn.shape[0])
    ]
    res = run_bass_kernel_spmd(
        nc, in_maps, core_ids=list(range(len(in_maps))), trace=trace
    )
    partials = np.array(
        [res.results[b]["out"][0, 0] for b in range(len(in_maps))], np.float32
    )
    return partials, res


def kernel(predicted_noise, actual_noise, predicted_points_coarse,
           target_points_coarse):
    partials, _ = run_cores(
        {
            "predicted_noise": predicted_noise,
            "actual_noise": actual_noise,
            "predicted_points_coarse": predicted_points_coarse,
            "target_points_coarse": target_points_coarse,
        }
    )
    return np.array(np.sum(partials, dtype=np.float32), dtype=np.float32)



# revision 2
# speedup vs baseline: 1.5697x; 1.5697x over previous
"""Trainium2 Bass kernel v2 for DiffusionLoss (L1 noise loss + chamfer distance).

Contract: kernel(**inputs) takes the FULL [8, 16384, 3] f32 inputs, shards the
batch across 8 NeuronCores (1 batch element per core), and returns the full
scalar loss (shape () float32). Host sums the 8 per-core partial scalars.

This target pays a large (~40us) dispatch cost per STATIC instruction, while
For_i hardware-loop iterations run at near-hardware speed (~30us/back-edge).
v2 therefore restructures the baseline's fully unrolled ~1.3k-instruction
stream into a small static program (~100 instrs) + For_i band loops.

Math: distances are built as full squared distances via the ScalarEngine:
  sq_d[p, j] = Square(b_d[j] * (-1) + a_d[i])    (ACT, per-partition bias)
  D[p, j]    = sq_x + sq_y + sq_z                (DVE bf16 adds, 2x mode)
so no |a|^2 / |b|^2 tiles, no cancellation (bf16-safe: D is exact-relative),
no relu (D >= 0 by construction), and the ACT engine runs in parallel with
the DVE adds / min-reductions. Per band t (128 pred points i = 128t + p):
  rm[p, t]     = min_j D[p, j]                  (row mins; summed at end)
  colacc[p, j] = min(colacc[p, j], D[p, j])     (col mins over bands;
                                                 partition-reduced at end)
Targets j are processed in two halves of 8192 to fit SBUF; b coords are
bf16-quantized (slightly perturbed target points — exact geometry on the
perturbed cloud, fine at the 2e-2 tolerance).
"""

import numpy as np
from contextlib import ExitStack

import concourse.bacc as bacc
import concourse.bass as bass
import concourse.bass_isa as bass_isa
import concourse.mybir as mybir
import concourse.tile as tile
from concourse.bass_utils import run_bass_kernel_spmd

F32 = mybir.dt.float32
BF16 = mybir.dt.bfloat16
A = mybir.AluOpType
AX = mybir.AxisListType
AF = mybir.ActivationFunctionType
ds = bass.ds

B = 8
N = 16384
NCORES = 8
P = 128
BIG = 3.0e38
JH = 8192          # j-half width
U = 4              # bands per For_i iteration

NOISE_WEIGHT = 1.0
CHAMFER_WEIGHT = 0.1


def diffusion_loss_kernel(ctx, tc, out_ap, ins, n=N, variant="full",
                          u=U, pairred=False, stagger=False):
    do_act = variant not in ("noact", "static")
    do_dve = variant not in ("nodve", "static")
    do_loop = variant != "static"
    nc = tc.nc
    assert n % P == 0
    nt = n // P            # number of i-bands
    jh = min(JH, n)
    nh = n // jh           # number of j-halves
    npp = n // P
    wn = float(NOISE_WEIGHT / (B * n * 3))
    wc = float(CHAMFER_WEIGHT / (B * n))
    u = min(u, nt)

    persist = ctx.enter_context(tc.tile_pool(name="persist", bufs=1))

    # ---------------- noise L1 loss ----------------
    noiseacc = persist.tile([P, 1], F32)
    with tc.tile_pool(name="noise", bufs=1) as nprep:
        pn_nat = nprep.tile([P, 3 * npp], F32)
        an_nat = nprep.tile([P, 3 * npp], F32)
        nc.sync.dma_start(pn_nat[:], ins["pn"].rearrange("(p f) d -> p (f d)", p=P))
        nc.sync.dma_start(an_nat[:], ins["an"].rearrange("(p f) d -> p (f d)", p=P))
        nc.vector.tensor_sub(pn_nat[:], pn_nat[:], an_nat[:])
        nc.vector.tensor_reduce(
            noiseacc[:], pn_nat[:], axis=AX.X, op=A.add, apply_absolute_value=True
        )

    # ---------------- persistent tiles ----------------
    # acols[p, d, t] = pred coord d of point i = 128*t + p
    acols = persist.tile([P, 3, nt], F32)
    nc.sync.dma_start(acols[:], ins["pred"].rearrange("(t p) d -> p d t", p=P))
    # ACT bias APs cannot take register (loop-var) offsets, so the bias
    # scalars for each iteration's u bands are staged to a fixed address
    # by the otherwise-idle Pool engine, one copy per iteration.
    stage = persist.tile([P, 3, u], F32)
    rm = persist.tile([P, nt], F32)          # per-band row mins
    rtmp2 = persist.tile([P, 2], F32)
    colacc = persist.tile([P, nh, jh], BF16)  # running col mins (j = h*jh + jj)
    nc.vector.memset(colacc.rearrange("p a b -> p (a b)"), BIG)
    b3 = persist.tile([P, 3, jh], BF16)       # b coords for current half, bcast

    with tc.tile_pool(name="main", bufs=1) as main:
        Aq = main.tile([P, 2, jh], BF16)      # D accumulator, band-parity dbuf
        Bq = main.tile([P, 3, jh], BF16)      # sq_y (parity dbuf) / sq_z
        if not (do_act and do_dve):
            nc.vector.memset(rm[:], 0.0)
            nc.vector.memset(Aq.rearrange("p a b -> p (a b)"), 1.0)
            nc.vector.memset(Bq.rearrange("p a b -> p (a b)"), 1.0)

        for h in range(nh):
            jb = h * jh
            # ---- load + bf16-cast + broadcast this half's target coords ----
            with tc.tile_pool(name="bprep", bufs=1) as bprep:
                scr = bprep.tile([1, jh], F32)
                for c in range(3):
                    nc.sync.dma_start(
                        scr[:],
                        ins["targ"][jb : jb + jh, c : c + 1].rearrange("j o -> o j"),
                    )
                    nc.vector.tensor_copy(b3[0:1, c, :], scr[:])
                nc.gpsimd.partition_broadcast(
                    b3.rearrange("p a b -> p (a b)"),
                    b3[0:1, :, :].rearrange("p a b -> p (a b)"),
                    channels=P,
                )

            # ---- band loop ----
            if not do_loop:
                continue
            with tc.For_i(0, nt, u, staggered_reset=stagger) as t0:
                # one Pool staging copy for all u bands' bias scalars
                nc.gpsimd.tensor_copy(stage[:, :, :], acols[:, :, ds(t0, u)])
                for k in range(u // 2):
                    for pi in (0, 1):
                        uu = 2 * k + pi
                        if do_act:
                            nc.scalar.activation(
                                Aq[:, pi, :], b3[:, 0, :], AF.Square,
                                bias=stage[:, 0, uu : uu + 1], scale=-1.0,
                            )
                            nc.scalar.activation(
                                Bq[:, pi, :], b3[:, 1, :], AF.Square,
                                bias=stage[:, 1, uu : uu + 1], scale=-1.0,
                            )
                            nc.scalar.activation(
                                Bq[:, 2, :], b3[:, 2, :], AF.Square,
                                bias=stage[:, 2, uu : uu + 1], scale=-1.0,
                            )
                        if do_dve:
                            nc.vector.tensor_add(
                                Aq[:, pi, :], Aq[:, pi, :], Bq[:, pi, :]
                            )
                            nc.vector.tensor_add(
                                Aq[:, pi, :], Aq[:, pi, :], Bq[:, 2, :]
                            )
                            nc.vector.tensor_tensor(
                                out=colacc[:, h, :], in0=colacc[:, h, :],
                                in1=Aq[:, pi, :], op=A.min,
                            )
                            if not pairred:
                                if h == 0:
                                    nc.vector.tensor_reduce(
                                        rm[:, ds(t0 + uu, 1)], Aq[:, pi, :],
                                        axis=AX.X, op=A.min,
                                    )
                                else:
                                    nc.vector.tensor_reduce(
                                        rtmp2[:, 0:1], Aq[:, pi, :],
                                        axis=AX.X, op=A.min,
                                    )
                                    nc.vector.tensor_tensor(
                                        out=rm[:, ds(t0 + uu, 1)],
                                        in0=rm[:, ds(t0 + uu, 1)],
                                        in1=rtmp2[:, 0:1], op=A.min,
                                    )
                    if do_dve and pairred:
                        # one 2-band row-min reduce per pair
                        if h == 0:
                            nc.vector.tensor_reduce(
                                rm[:, ds(t0 + 2 * k, 2)], Aq[:, :, :],
                                axis=AX.X, op=A.min,
                            )
                        else:
                            nc.vector.tensor_reduce(
                                rtmp2[:], Aq[:, :, :], axis=AX.X, op=A.min
                            )
                            nc.vector.tensor_tensor(
                                out=rm[:, ds(t0 + 2 * k, 2)],
                                in0=rm[:, ds(t0 + 2 * k, 2)],
                                in1=rtmp2[:], op=A.min,
                            )

        # ---------------- epilogue ----------------
        # col mins: negate, partition-max, sum
        negc = Bq[:, 0:2, :].rearrange("p a b -> p (a b)")
        nc.vector.tensor_scalar_mul(
            negc, colacc.rearrange("p a b -> p (a b)"), -1.0
        )
        posm = Aq.rearrange("p a b -> p (a b)")
        nc.gpsimd.partition_all_reduce(
            posm, negc, channels=P, reduce_op=bass_isa.ReduceOp.max
        )
        csum = persist.tile([1, 1], F32)
        nc.vector.tensor_reduce(
            csum[:], Aq[0:1, :, :].rearrange("p a b -> p (a b)"),
            axis=AX.X, op=A.add,
        )

        # row mins: sum over bands
        racc = persist.tile([P, 1], F32)
        nc.vector.tensor_reduce(racc[:], rm[:], axis=AX.X, op=A.add)

        # combine: per-partition v = noise*wn + rowsum*wc, then partition-sum
        v1 = persist.tile([P, 1], F32)
        nc.vector.tensor_scalar_mul(v1[:], noiseacc[:], wn)
        v = persist.tile([P, 1], F32)
        nc.vector.scalar_tensor_tensor(
            out=v[:], in0=racc[:], scalar=wc, in1=v1[:], op0=A.mult, op1=A.add
        )
        vv = persist.tile([P, 1], F32)
        nc.gpsimd.partition_all_reduce(
            vv[:], v[:], channels=P, reduce_op=bass_isa.ReduceOp.add
        )
        # csum holds sum_j(-colmin_j): out = vv - wc*csum... (-wc)*csum + vv
        fs = persist.tile([1, 1], F32)
        nc.vector.scalar_tensor_tensor(
            out=fs[:], in0=csum[:], scalar=-wc, in1=vv[0:1, :],
            op0=A.mult, op1=A.add,
        )
        nc.sync.dma_start(out_ap, fs[:])


_CACHE = {}


def build_program(n=N, variant="full", u=U, pairred=False, stagger=False):
    key = (n, variant, u, pairred, stagger)
    if key not in _CACHE:
        nc = bacc.Bacc(
            "TRN2", target_bir_lowering=False, debug=False, enable_asserts=False
        )
        ins = {
            name: nc.dram_tensor(name, [n, 3], F32, kind="ExternalInput").ap()
            for name in ("pn", "an", "pred", "targ")
        }
        out_ap = nc.dram_tensor("out", [1, 1], F32, kind="ExternalOutput").ap()
        with tile.TileContext(nc) as tc:
            with ExitStack() as ctx:
                diffusion_loss_kernel(ctx, tc, out_ap, ins, n=n, variant=variant,
                                      u=u, pairred=pairred, stagger=stagger)
        nc.compile()
        _CACHE[key] = nc
    return _CACHE[key]


def run_cores(inputs, n=N, trace=False):
    """Run the SPMD program over the batch; returns (partials, results)."""
    nc = build_program(n=n)
    pn = np.ascontiguousarray(np.asarray(inputs["predicted_noise"], np.float32))
    an = np.ascontiguousarray(np.asarray(inputs["actual_noise"], np.float32))
    pred = np.ascontiguousarray(
        np.asarray(inputs["predicted_points_coarse"], np.float32)
    )
    targ = np.ascontiguousarray(
        np.asarray(inputs["target_points_coarse"], np.float32)
    )
    in_maps = [
        {"pn": pn[b], "an": an[b], "pred": pred[b], "targ": targ[b]}
        for b in range(pn.shape[0])
    ]
    res = run_bass_kernel_spmd(
        nc, in_maps, core_ids=list(range(len(in_maps))), trace=trace
    )
    partials = np.array(
        [res.results[b]["out"][0, 0] for b in range(len(in_maps))], np.float32
    )
    return partials, res


def kernel(predicted_noise, actual_noise, predicted_points_coarse,
           target_points_coarse):
    partials, _ = run_cores(
        {
            "predicted_noise": predicted_noise,
            "actual_noise": actual_noise,
            "predicted_points_coarse": predicted_points_coarse,
            "target_points_coarse": target_points_coarse,
        }
    )
    return np.array(np.sum(partials, dtype=np.float32), dtype=np.float32)
